# revision 13
# baseline (speedup 1.0000x reference)
"""Trainium2 Bass kernel for AdaptiveSparseCrossAttention.

Reference math (B=2, N=2048, C=1024, H=16, Dh=64):
    q  = (x1 @ Wq) [B,H,N,Dh];  k,v = (x2 @ Wkv) [B,H,N,Dh]
    S  = (q * Dh^-0.5) @ k^T                  [B,H,N,N]
    P  = wn0 * softmax(S) + wn1 * relu(S)^2   (wn = softmax(w))
    out = (P @ v).reshape(B,N,C) @ Wproj + bproj

Sharding: 32 (batch, head) pairs over 8 cores -> core i handles batch
b=i//4, heads 4g..4g+3 with g=i%4.  Each core computes a partial
projection [2048,1024]; a ReduceScatter(add) over the 4 cores of each
batch yields 512 distinct output rows per core; the host concatenates.

Device-side layout (per core), all matmuls in (128,128) array mode:
    qT/kT per head: [128, 2048] fp16, Dh values zero-padded to 128 parts
    S^T tile  = kT_slice.T @ qT_chunk   -> PSUM [128 ktoks, 512 q] fp32
    E = exp(S^T) (ScalarE), R2 = (S max 0)*S (VectorE STT) -> fp16 SBUF
    O1 += v_aug.T @ E (v_aug has a ones column -> row 64 = softmax denom)
    O2 += v_aug.T @ R2
    O_hT = (O1 * wn0/rowsum-broadcast) + wn1*O2   (per-head [64, q])
    partial = bias_bcast + sum_h O_hT.T @ Wproj_rows(h)
"""

import os
import numpy as np

import concourse.bass as bass
import concourse.tile as tile
from concourse import bacc, mybir
from concourse.bass_utils import run_bass_kernel_spmd

F16 = mybir.dt.float16
F32 = mybir.dt.float32

B, N, C, H, Dh = 2, 2048, 1024, 16, 64
NCORES = 8
HPC = 4            # heads per core
GROUPS = [[0, 1, 2, 3], [4, 5, 6, 7]]
CHUNK = 512        # q-span processed per (head, chunk) pass
NKT = N // 128     # 16 k-token tiles
NCH = N // CHUNK   # 4 q-chunks
KTG = 2            # k-tiles per S-psum group (exp/relu2 FD = KTG*CHUNK)

RELU2_STT = os.environ.get("K_RELU2", "stt") == "stt"

_CACHE = {}


def _build(wn0: float, wn1: float):
    nc = bacc.Bacc(
        "TRN2", target_bir_lowering=False, debug=False, num_devices=NCORES
    )

    # ---- DRAM parameters (per-core shards fed via in_maps) ----
    x1t = nc.dram_tensor("x1t", [C, N], F16, kind="ExternalInput").ap()
    x2t = nc.dram_tensor("x2t", [C, N], F16, kind="ExternalInput").ap()
    wq = nc.dram_tensor("wq", [C, HPC * Dh], F16, kind="ExternalInput").ap()
    wk = nc.dram_tensor("wk", [C, HPC * Dh], F16, kind="ExternalInput").ap()
    wv = nc.dram_tensor("wv", [C, HPC * Dh], F16, kind="ExternalInput").ap()
    wp = nc.dram_tensor("wp", [HPC, 128, C], F16, kind="ExternalInput").ap()
    biasp = nc.dram_tensor("biasp", [128, C], F16, kind="ExternalInput").ap()
    c_one = nc.dram_tensor("c_one", [128, 128], F16, kind="ExternalInput").ap()
    c_wn0 = nc.dram_tensor("c_wn0", [128, 65], F16, kind="ExternalInput").ap()
    out_ext = nc.dram_tensor(
        "out", [N // 4, C], F32, kind="ExternalOutput"
    ).ap()

    with tile.TileContext(nc) as tc:
        from contextlib import ExitStack

        with ExitStack() as ctx:
            consts = ctx.enter_context(tc.tile_pool(name="consts", bufs=1))
            wpool = ctx.enter_context(tc.tile_pool(name="wpool", bufs=1))
            qkpool = ctx.enter_context(tc.tile_pool(name="qkpool", bufs=1))
            vpool = ctx.enter_context(tc.tile_pool(name="vpool", bufs=1))
            opool = ctx.enter_context(tc.tile_pool(name="opool", bufs=1))
            dram = ctx.enter_context(
                tc.tile_pool(name="dram", bufs=1, space="DRAM")
            )

            ps_s = ctx.enter_context(
                tc.tile_pool(name="ps_s", bufs=2, space="PSUM")
            )
            ps_o1 = ctx.enter_context(
                tc.tile_pool(name="ps_o1", bufs=1, space="PSUM")
            )
            ps_o2 = ctx.enter_context(
                tc.tile_pool(name="ps_o2", bufs=1, space="PSUM")
            )
            ps_rb = ctx.enter_context(
                tc.tile_pool(name="ps_rb", bufs=1, space="PSUM")
            )
            ps_pp = ctx.enter_context(
                tc.tile_pool(name="ps_pp", bufs=1, space="PSUM")
            )

            # ---- persistent SBUF tensors ----
            one_s = consts.tile([128, 128], F16, tag="one")
            wn0_s = consts.tile([128, 65], F16, tag="wn0")
            bias_s = consts.tile([128, C], F16, tag="bias")
            nc.sync.dma_start(out=one_s[:], in_=c_one[:])
            nc.sync.dma_start(out=wn0_s[:], in_=c_wn0[:])
            nc.sync.dma_start(out=bias_s[:], in_=biasp[:])

            wq_s = [wpool.tile([128, HPC * Dh], F16, tag=f"wq{k}", name=f"wq{k}") for k in range(8)]
            wk_s = [wpool.tile([128, HPC * Dh], F16, tag=f"wk{k}", name=f"wk{k}") for k in range(8)]
            wv_s = [wpool.tile([128, HPC * Dh], F16, tag=f"wv{k}", name=f"wv{k}") for k in range(8)]
            wp_s = [wpool.tile([128, C], F16, tag=f"wp{h}", name=f"wp{h}") for h in range(HPC)]
            for k in range(8):
                sl = slice(k * 128, (k + 1) * 128)
                nc.sync.dma_start(out=wq_s[k][:], in_=wq[sl, :])
                nc.sync.dma_start(out=wk_s[k][:], in_=wk[sl, :])
                nc.sync.dma_start(out=wv_s[k][:], in_=wv[sl, :])
            for h in range(HPC):
                nc.sync.dma_start(out=wp_s[h][:], in_=wp[h, :, :])

            # per-head q^T / k^T, zero-padded to K=128
            qT = [qkpool.tile([128, N], F16, tag=f"qT{h}", name=f"qT{h}") for h in range(HPC)]
            kT = [qkpool.tile([128, N], F16, tag=f"kT{h}", name=f"kT{h}") for h in range(HPC)]
            for h in range(HPC):
                nc.vector.memset(qT[h][:], 0.0)
                nc.vector.memset(kT[h][:], 0.0)

            # v with ones column: [128 toks, HPC, 65]
            v_s = [vpool.tile([128, HPC, 65], F16, tag=f"v{t}", name=f"v{t}") for t in range(NKT)]
            for t in range(NKT):
                nc.vector.memset(v_s[t][:, :, 64:65], 1.0)

            # per-head O^T accumulators [128(64 dh + 64 zeros), N] fp16
            oT = [opool.tile([128, N], F16, tag=f"oT{h}", name=f"oT{h}") for h in range(HPC)]
            for h in range(HPC):
                nc.vector.memset(oT[h][:], 0.0)

            # per-chunk partial + reduce-scatter buffers so the collective
            # for chunk c overlaps compute of chunk c+1
            part_ds = [
                dram.tile([CHUNK, C], F32, name=f"part_d{c}") for c in range(NCH)
            ]
            rs_ds = [
                dram.tile([CHUNK // 4, C], F32, name=f"rs_d{c}")
                for c in range(NCH)
            ]

            # ---- Phase 1: QKV projections ----
            with tc.tile_pool(name="xt", bufs=1) as xpool:
                x1_s = [xpool.tile([128, N], F16, tag=f"x1_{k}", name=f"x1_{k}") for k in range(8)]
                x2_s = [xpool.tile([128, N], F16, tag=f"x2_{k}", name=f"x2_{k}") for k in range(8)]
                for k in range(8):
                    sl = slice(k * 128, (k + 1) * 128)
                    nc.sync.dma_start(out=x1_s[k][:], in_=x1t[sl, :])
                    nc.sync.dma_start(out=x2_s[k][:], in_=x2t[sl, :])

                # qT / kT:  out[h-pair 128, nq 512] = Wq_slice.T @ x1t
                for which, w_s, x_s, dst in (
                    ("q", wq_s, x1_s, qT),
                    ("k", wk_s, x2_s, kT),
                ):
                    for m in range(2):  # head pair (2m, 2m+1)
                        for n in range(4):  # 512-wide q spans
                            pt = ps_s.tile([128, KTG, CHUNK], F32, tag="s")
                            acc = pt[:, 0, :]
                            for k in range(8):
                                nc.tensor.matmul(
                                    acc,
                                    lhsT=w_s[k][:, m * 128 : (m + 1) * 128],
                                    rhs=x_s[k][:, n * 512 : (n + 1) * 512],
                                    start=(k == 0),
                                    stop=(k == 7),
                                )
                            span = slice(n * 512, (n + 1) * 512)
                            nc.scalar.copy(
                                out=dst[2 * m][0:64, span], in_=acc[0:64, :]
                            )
                            nc.scalar.copy(
                                out=dst[2 * m + 1][64:128, span],
                                in_=acc[64:128, :],
                            )

                # v: out[tok 128, HPC*Dh] = x2t_slice.T @ Wv
                for t in range(NKT):
                    pt = ps_s.tile([128, KTG, CHUNK], F32, tag="s")
                    acc = pt[:, 0, 0:256]
                    for k in range(8):
                        nc.tensor.matmul(
                            acc,
                            lhsT=x2_s[k][:, t * 128 : (t + 1) * 128],
                            rhs=wv_s[k][:, 0:256],
                            start=(k == 0),
                            stop=(k == 7),
                        )
                    nc.vector.tensor_copy(
                        out=v_s[t][:, :, 0:64],
                        in_=acc.rearrange("p (h d) -> p h d", h=HPC),
                    )

            # ---- Phase 2: attention + blend per (chunk, head) ----
            epool = ctx.enter_context(tc.tile_pool(name="epool", bufs=2))
            r2pool = ctx.enter_context(tc.tile_pool(name="r2pool", bufs=2))
            blpool = ctx.enter_context(tc.tile_pool(name="blpool", bufs=2))
            rspool = ctx.enter_context(tc.tile_pool(name="rspool", bufs=1))
            pspool = ctx.enter_context(tc.tile_pool(name="pspool", bufs=2))

            rsinv = rspool.tile([128, CHUNK], F16, tag="rsinv")
            nc.vector.memset(rsinv[:], 0.0)

            for c in range(NCH):
                qspan = slice(c * CHUNK, (c + 1) * CHUNK)
                for h in range(HPC):
                    e_t = epool.tile([128, NKT, CHUNK], F16, tag="e")
                    r2_t = r2pool.tile([128, NKT, CHUNK], F16, tag="r2")

                    for g in range(NKT // KTG):
                        s_ps = ps_s.tile([128, KTG, CHUNK], F32, tag="s")
                        for j in range(KTG):
                            kt = g * KTG + j
                            nc.tensor.matmul(
                                s_ps[:, j, :],
                                lhsT=kT[h][:, kt * 128 : (kt + 1) * 128],
                                rhs=qT[h][:, qspan],
                                start=True,
                                stop=True,
                            )
                        gs = slice(g * KTG, (g + 1) * KTG)
                        nc.scalar.activation(
                            out=e_t[:, gs, :],
                            in_=s_ps[:],
                            func=mybir.ActivationFunctionType.Exp,
                        )
                        # relu(S)^2: max into SBUF fp16, then square in place
                        # (walrus forbids two PSUM operands on one DVE op).
                        # Every 3rd group's relu runs on ScalarE to balance
                        # the engines (DVE reads fp32 PSUM at 1x).
                        if g % 3 == 2:
                            nc.scalar.activation(
                                out=r2_t[:, gs, :],
                                in_=s_ps[:],
                                func=mybir.ActivationFunctionType.Relu,
                            )
                        else:
                            nc.vector.tensor_scalar_max(
                                out=r2_t[:, gs, :], in0=s_ps[:], scalar1=0.0
                            )
                        nc.vector.tensor_mul(
                            out=r2_t[:, gs, :],
                            in0=r2_t[:, gs, :],
                            in1=r2_t[:, gs, :],
                        )

                    # interleave the two PV streams so consecutive matmuls
                    # share the same stationary operand (v_aug slice)
                    o1 = ps_o1.tile([128, CHUNK], F32, tag="o1")
                    o2 = ps_o2.tile([128, CHUNK], F32, tag="o2")
                    for kt in range(NKT):
                        nc.tensor.matmul(
                            o1[0:65, :],
                            lhsT=v_s[kt][:, h, :],
                            rhs=e_t[:, kt, :],
                            start=(kt == 0),
                            stop=(kt == NKT - 1),
                        )
                        nc.tensor.matmul(
                            o2[0:65, :],
                            lhsT=v_s[kt][:, h, :],
                            rhs=r2_t[:, kt, :],
                            start=(kt == 0),
                            stop=(kt == NKT - 1),
                        )

                    # blend: oT[h] = (wn0/rowsum) * O1 + wn1 * O2
                    with nc.allow_low_precision(
                        reason="1/rowsum ~5e-4, fp16 rel eps is plenty"
                    ):
                        nc.vector.reciprocal(
                            out=rsinv[64:65, :], in_=o1[64:65, :]
                        )
                    rb = ps_rb.tile([128, CHUNK], F32, tag="rb")
                    nc.tensor.matmul(
                        rb[0:65, :],
                        lhsT=wn0_s[:],
                        rhs=rsinv[:],
                        start=True,
                        stop=True,
                    )
                    rb_sb = blpool.tile([128, CHUNK], F16, tag="rb_sb")
                    nc.scalar.copy(out=rb_sb[0:64, :], in_=rb[0:64, :])
                    xb = blpool.tile([128, CHUNK], F16, tag="xb")
                    nc.vector.tensor_mul(
                        out=xb[0:64, :], in0=o1[0:64, :], in1=rb_sb[0:64, :]
                    )
                    nc.vector.scalar_tensor_tensor(
                        out=oT[h][0:64, qspan],
                        in0=o2[0:64, :],
                        scalar=float(wn1),
                        in1=xb[0:64, :],
                        op0=mybir.AluOpType.mult,
                        op1=mybir.AluOpType.add,
                    )

                # ---- projection for this chunk ----
                for qt in range(CHUNK // 128):
                    row0 = c * CHUNK + qt * 128
                    part_sb = pspool.tile([128, C], F32, tag="part")
                    for cc in range(2):
                        csl = slice(cc * 512, (cc + 1) * 512)
                        pp = ps_pp.tile([128, 512], F32, tag="pp")
                        nc.tensor.matmul(
                            pp[:],
                            lhsT=one_s[:],
                            rhs=bias_s[:, csl],
                            start=True,
                            stop=False,
                        )
                        for h in range(HPC):
                            nc.tensor.matmul(
                                pp[:],
                                lhsT=oT[h][:, row0 : row0 + 128],
                                rhs=wp_s[h][:, csl],
                                start=False,
                                stop=(h == HPC - 1),
                            )
                        nc.scalar.copy(out=part_sb[:, csl], in_=pp[:])
                    nc.sync.dma_start(
                        out=part_ds[c][qt * 128 : (qt + 1) * 128, :],
                        in_=part_sb[:],
                    )

                # reduce this chunk across the batch group; overlaps with
                # compute of the next chunk.  Core rank r receives global
                # rows c*CHUNK + r*(CHUNK//4) + [0, CHUNK//4).
                nc.gpsimd.collective_compute(
                    "ReduceScatter",
                    mybir.AluOpType.add,
                    replica_groups=GROUPS,
                    ins=[part_ds[c].opt()],
                    outs=[rs_ds[c].opt()],
                )
                qq = CHUNK // 4
                nc.sync.dma_start(
                    out=out_ext[c * qq : (c + 1) * qq, :], in_=rs_ds[c][:]
                )

    nc.compile()
    return nc


def _ensure_profile_hook():
    """The container's antenv lacks axon_hooks; recreate it and register
    the ctypes NTFF hook so trace=True yields neuron-profile exec times."""
    import sys
    import types

    try:
        from antenv import axon_hooks  # noqa: F401
    except ImportError:
        import antenv

        mod = types.ModuleType("antenv.axon_hooks")
        _hook = [None]
        mod.set_axon_ntff_profile_hook = lambda h: _hook.__setitem__(0, h)
        mod.get_axon_ntff_profile_hook = lambda: _hook[0]
        sys.modules["antenv.axon_hooks"] = mod
        antenv.axon_hooks = mod
        try:
            from trn_agent_boot.trn_boot import _ntff_profile_via_ctypes

            mod.set_axon_ntff_profile_hook(
                _ntff_profile_via_ctypes("/opt/axon/libaxon_pjrt.so")
            )
        except Exception as e:  # pragma: no cover
            print(f"[kernel] NTFF hook registration failed: {e}")
    # keep profiling artifacts local; the S3 upload has no creds here
    import concourse.bass_utils as bu

    bu.upload_artifacts = lambda tmpdir: tmpdir


def _softmax2(w):
    w = np.asarray(w, np.float64)
    e = np.exp(w - w.max())
    e /= e.sum()
    return float(e[0]), float(e[1])


def kernel(x1, x2, Wq, Wkv, Wproj, bproj, w):
    x1 = np.asarray(x1, np.float32)
    x2 = np.asarray(x2, np.float32)
    Wq = np.asarray(Wq, np.float32)
    Wkv = np.asarray(Wkv, np.float32)
    Wproj = np.asarray(Wproj, np.float32)
    bproj = np.asarray(bproj, np.float32)
    wn0, wn1 = _softmax2(w)

    key = (round(wn0, 9), round(wn1, 9))
    if key not in _CACHE:
        _CACHE[key] = _build(wn0, wn1)
    nc = _CACHE[key]

    scale = Dh ** -0.5
    c_one = np.ones((128, 128), np.float16)
    c_wn0 = np.full((128, 65), wn0, np.float16)

    in_maps = []
    for core in range(NCORES):
        b, g = divmod(core, HPC)
        cols = slice(g * HPC * Dh, (g + 1) * HPC * Dh)
        wp_pad = np.zeros((HPC, 128, C), np.float16)
        for h in range(HPC):
            r0 = g * HPC * Dh + h * Dh
            wp_pad[h, 0:64, :] = Wproj[r0 : r0 + Dh, :].astype(np.float16)
        bias_i = np.zeros((128, C), np.float16)
        if g == 0:
            bias_i[0, :] = bproj.astype(np.float16)
        in_maps.append(
            {
                "x1t": np.ascontiguousarray(x1[b].T).astype(np.float16),
                "x2t": np.ascontiguousarray(x2[b].T).astype(np.float16),
                "wq": (Wq[:, cols] * scale).astype(np.float16),
                "wk": Wkv[:, 0:C][:, cols].astype(np.float16),
                "wv": Wkv[:, C : 2 * C][:, cols].astype(np.float16),
                "wp": wp_pad,
                "biasp": bias_i,
                "c_one": c_one,
                "c_wn0": c_wn0,
            }
        )

    bench = os.environ.get("K_BENCH", "0") == "1"
    if bench:
        _ensure_profile_hook()
    res = run_bass_kernel_spmd(
        nc, in_maps, core_ids=list(range(NCORES)), trace=bench
    )
    if bench:
        kernel.last_exec_ns = res.exec_time_ns
        kernel.last_trace = (
            res.instructions_and_trace[1] if res.instructions_and_trace else None
        )

    full = np.empty((B, N, C), np.float32)
    qq = CHUNK // 4
    for b in range(B):
        for r in range(4):
            o = res.results[4 * b + r]["out"]
            for c in range(NCH):
                full[b, c * CHUNK + r * qq : c * CHUNK + (r + 1) * qq, :] = o[
                    c * qq : (c + 1) * qq, :
                ]
    return full


kernel.last_exec_ns = None
kernel.last_trace = None


# revision 14
# speedup vs baseline: 1.0978x; 1.0978x over previous
"""Trainium2 Bass kernel for AdaptiveSparseCrossAttention.

Reference math (B=2, N=2048, C=1024, H=16, Dh=64):
    q  = (x1 @ Wq) [B,H,N,Dh];  k,v = (x2 @ Wkv) [B,H,N,Dh]
    S  = (q * Dh^-0.5) @ k^T                  [B,H,N,N]
    P  = wn0 * softmax(S) + wn1 * relu(S)^2   (wn = softmax(w))
    out = (P @ v).reshape(B,N,C) @ Wproj + bproj

Sharding: 32 (batch, head) pairs over 8 cores -> core i handles batch
b=i//4, heads 4g..4g+3 with g=i%4.  Each core computes a partial
projection [2048,1024]; a ReduceScatter(add) over the 4 cores of each
batch yields 512 distinct output rows per core; the host concatenates.

Device-side layout (per core), all matmuls in (128,128) array mode:
    qT/kT per head: [128, 2048] fp16, Dh values zero-padded to 128 parts
    S^T tile  = kT_slice.T @ qT_chunk   -> PSUM [128 ktoks, 512 q] fp32
    E = exp(S^T) (ScalarE), R2 = (S max 0)*S (VectorE STT) -> fp16 SBUF
    O1 += v_aug.T @ E (v_aug has a ones column -> row 64 = softmax denom)
    O2 += v_aug.T @ R2
    O_hT = (O1 * wn0/rowsum-broadcast) + wn1*O2   (per-head [64, q])
    partial = bias_bcast + sum_h O_hT.T @ Wproj_rows(h)
"""

import os
import numpy as np

import concourse.bass as bass
import concourse.tile as tile
from concourse import bacc, mybir
from concourse.bass_utils import run_bass_kernel_spmd

F16 = mybir.dt.float16
F32 = mybir.dt.float32

B, N, C, H, Dh = 2, 2048, 1024, 16, 64
NCORES = 8
HPC = 4            # heads per core
GROUPS = [[0, 1, 2, 3], [4, 5, 6, 7]]
CHUNK = 512        # q-span processed per (head, chunk) pass
NKT = N // 128     # 16 k-token tiles
NCH = N // CHUNK   # 4 q-chunks
KTG = 2            # k-tiles per S-psum group (exp/relu2 FD = KTG*CHUNK)

RELU2_STT = os.environ.get("K_RELU2", "stt") == "stt"

_CACHE = {}


def _build(wn0: float, wn1: float):
    nc = bacc.Bacc(
        "TRN2", target_bir_lowering=False, debug=False, num_devices=NCORES
    )

    # ---- DRAM parameters (per-core shards fed via in_maps) ----
    x1t = nc.dram_tensor("x1t", [C, N], F16, kind="ExternalInput").ap()
    x2t = nc.dram_tensor("x2t", [C, N], F16, kind="ExternalInput").ap()
    wq = nc.dram_tensor("wq", [C, HPC * Dh], F16, kind="ExternalInput").ap()
    wk = nc.dram_tensor("wk", [C, HPC * Dh], F16, kind="ExternalInput").ap()
    wv = nc.dram_tensor("wv", [C, HPC * Dh], F16, kind="ExternalInput").ap()
    wp = nc.dram_tensor("wp", [HPC, 128, C], F16, kind="ExternalInput").ap()
    biasp = nc.dram_tensor("biasp", [128, C], F16, kind="ExternalInput").ap()
    c_one = nc.dram_tensor("c_one", [128, 128], F16, kind="ExternalInput").ap()
    c_wn0 = nc.dram_tensor("c_wn0", [128, 65], F16, kind="ExternalInput").ap()
    out_ext = nc.dram_tensor(
        "out", [N // 4, C], F16, kind="ExternalOutput"
    ).ap()

    with tile.TileContext(nc) as tc:
        from contextlib import ExitStack

        with ExitStack() as ctx:
            consts = ctx.enter_context(tc.tile_pool(name="consts", bufs=1))
            wpool = ctx.enter_context(tc.tile_pool(name="wpool", bufs=1))
            qkpool = ctx.enter_context(tc.tile_pool(name="qkpool", bufs=1))
            vpool = ctx.enter_context(tc.tile_pool(name="vpool", bufs=1))
            opool = ctx.enter_context(tc.tile_pool(name="opool", bufs=1))
            dram = ctx.enter_context(
                tc.tile_pool(name="dram", bufs=1, space="DRAM")
            )

            ps_s = ctx.enter_context(
                tc.tile_pool(name="ps_s", bufs=2, space="PSUM")
            )
            ps_o1 = ctx.enter_context(
                tc.tile_pool(name="ps_o1", bufs=1, space="PSUM")
            )
            ps_o2 = ctx.enter_context(
                tc.tile_pool(name="ps_o2", bufs=1, space="PSUM")
            )
            ps_rb = ctx.enter_context(
                tc.tile_pool(name="ps_rb", bufs=1, space="PSUM")
            )
            ps_pp = ctx.enter_context(
                tc.tile_pool(name="ps_pp", bufs=1, space="PSUM")
            )

            # ---- persistent SBUF tensors ----
            one_s = consts.tile([128, 128], F16, tag="one")
            wn0_s = consts.tile([128, 65], F16, tag="wn0")
            bias_s = consts.tile([128, C], F16, tag="bias")
            nc.sync.dma_start(out=one_s[:], in_=c_one[:])
            nc.sync.dma_start(out=wn0_s[:], in_=c_wn0[:])
            nc.sync.dma_start(out=bias_s[:], in_=biasp[:])

            wq_s = [wpool.tile([128, HPC * Dh], F16, tag=f"wq{k}", name=f"wq{k}") for k in range(8)]
            wk_s = [wpool.tile([128, HPC * Dh], F16, tag=f"wk{k}", name=f"wk{k}") for k in range(8)]
            wv_s = [wpool.tile([128, HPC * Dh], F16, tag=f"wv{k}", name=f"wv{k}") for k in range(8)]
            wp_s = [wpool.tile([128, C], F16, tag=f"wp{h}", name=f"wp{h}") for h in range(HPC)]
            for k in range(8):
                sl = slice(k * 128, (k + 1) * 128)
                nc.sync.dma_start(out=wq_s[k][:], in_=wq[sl, :])
                nc.sync.dma_start(out=wk_s[k][:], in_=wk[sl, :])
                nc.sync.dma_start(out=wv_s[k][:], in_=wv[sl, :])
            for h in range(HPC):
                nc.sync.dma_start(out=wp_s[h][:], in_=wp[h, :, :])

            # per-head q^T / k^T, zero-padded to K=128
            qT = [qkpool.tile([128, N], F16, tag=f"qT{h}", name=f"qT{h}") for h in range(HPC)]
            kT = [qkpool.tile([128, N], F16, tag=f"kT{h}", name=f"kT{h}") for h in range(HPC)]
            for h in range(HPC):
                nc.vector.memset(qT[h][:], 0.0)
                nc.vector.memset(kT[h][:], 0.0)

            # v with ones column: [128 toks, HPC, 65]
            v_s = [vpool.tile([128, HPC, 65], F16, tag=f"v{t}", name=f"v{t}") for t in range(NKT)]
            for t in range(NKT):
                nc.vector.memset(v_s[t][:, :, 64:65], 1.0)

            # per-head O^T accumulators [128(64 dh + 64 zeros), N] fp16
            oT = [opool.tile([128, N], F16, tag=f"oT{h}", name=f"oT{h}") for h in range(HPC)]
            for h in range(HPC):
                nc.vector.memset(oT[h][:], 0.0)

            partial_d = dram.tile([N, C], F16, name="partial_d")
            rs_d = dram.tile([N // 4, C], F16, name="rs_d")

            # ---- Phase 1: QKV projections ----
            with tc.tile_pool(name="xt", bufs=1) as xpool:
                x1_s = [xpool.tile([128, N], F16, tag=f"x1_{k}", name=f"x1_{k}") for k in range(8)]
                x2_s = [xpool.tile([128, N], F16, tag=f"x2_{k}", name=f"x2_{k}") for k in range(8)]
                for k in range(8):
                    sl = slice(k * 128, (k + 1) * 128)
                    nc.sync.dma_start(out=x1_s[k][:], in_=x1t[sl, :])
                    nc.sync.dma_start(out=x2_s[k][:], in_=x2t[sl, :])

                # qT / kT:  out[h-pair 128, nq 512] = Wq_slice.T @ x1t
                for which, w_s, x_s, dst in (
                    ("q", wq_s, x1_s, qT),
                    ("k", wk_s, x2_s, kT),
                ):
                    for m in range(2):  # head pair (2m, 2m+1)
                        for n in range(4):  # 512-wide q spans
                            pt = ps_s.tile([128, KTG, CHUNK], F32, tag="s")
                            acc = pt[:, 0, :]
                            for k in range(8):
                                nc.tensor.matmul(
                                    acc,
                                    lhsT=w_s[k][:, m * 128 : (m + 1) * 128],
                                    rhs=x_s[k][:, n * 512 : (n + 1) * 512],
                                    start=(k == 0),
                                    stop=(k == 7),
                                )
                            span = slice(n * 512, (n + 1) * 512)
                            nc.scalar.copy(
                                out=dst[2 * m][0:64, span], in_=acc[0:64, :]
                            )
                            nc.scalar.copy(
                                out=dst[2 * m + 1][64:128, span],
                                in_=acc[64:128, :],
                            )

                # v: out[tok 128, HPC*Dh] = x2t_slice.T @ Wv
                for t in range(NKT):
                    pt = ps_s.tile([128, KTG, CHUNK], F32, tag="s")
                    acc = pt[:, 0, 0:256]
                    for k in range(8):
                        nc.tensor.matmul(
                            acc,
                            lhsT=x2_s[k][:, t * 128 : (t + 1) * 128],
                            rhs=wv_s[k][:, 0:256],
                            start=(k == 0),
                            stop=(k == 7),
                        )
                    nc.vector.tensor_copy(
                        out=v_s[t][:, :, 0:64],
                        in_=acc.rearrange("p (h d) -> p h d", h=HPC),
                    )

            # ---- Phase 2: attention + blend, software-pipelined ----
            # S/exp/relu2 of step i+1 are issued before PV/blend of step i,
            # so ScalarE/VectorE chew the next head's scores while the PE
            # runs the current head's PV matmuls.
            epool = ctx.enter_context(tc.tile_pool(name="epool", bufs=2))
            r2pool = ctx.enter_context(tc.tile_pool(name="r2pool", bufs=2))
            blpool = ctx.enter_context(tc.tile_pool(name="blpool", bufs=2))
            rspool = ctx.enter_context(tc.tile_pool(name="rspool", bufs=1))
            pspool = ctx.enter_context(tc.tile_pool(name="pspool", bufs=2))

            rsinv = rspool.tile([128, CHUNK], F16, tag="rsinv")
            nc.vector.memset(rsinv[:], 0.0)

            def do_scores(c, h):
                """S^T matmuls + exp + relu^2 for (chunk c, head h)."""
                qspan = slice(c * CHUNK, (c + 1) * CHUNK)
                e_t = epool.tile([128, NKT, CHUNK], F16, tag="e", name=f"e{c}_{h}")
                r2_t = r2pool.tile(
                    [128, NKT, CHUNK], F16, tag="r2", name=f"r2{c}_{h}"
                )
                for g in range(NKT // KTG):
                    s_ps = ps_s.tile(
                        [128, KTG, CHUNK], F32, tag="s", name=f"s{c}_{h}_{g}"
                    )
                    for j in range(KTG):
                        kt = g * KTG + j
                        nc.tensor.matmul(
                            s_ps[:, j, :],
                            lhsT=kT[h][:, kt * 128 : (kt + 1) * 128],
                            rhs=qT[h][:, qspan],
                            start=True,
                            stop=True,
                        )
                    gs = slice(g * KTG, (g + 1) * KTG)
                    nc.scalar.activation(
                        out=e_t[:, gs, :],
                        in_=s_ps[:],
                        func=mybir.ActivationFunctionType.Exp,
                    )
                    # relu(S)^2: max into SBUF fp16, then square in place
                    # (walrus forbids two PSUM operands on one DVE op).
                    # Some groups' relu runs on ScalarE to balance engines
                    # (DVE reads fp32 PSUM at 1x).
                    if g % 3 == 2:
                        nc.scalar.activation(
                            out=r2_t[:, gs, :],
                            in_=s_ps[:],
                            func=mybir.ActivationFunctionType.Relu,
                        )
                    else:
                        nc.vector.tensor_scalar_max(
                            out=r2_t[:, gs, :], in0=s_ps[:], scalar1=0.0
                        )
                    nc.vector.tensor_mul(
                        out=r2_t[:, gs, :],
                        in0=r2_t[:, gs, :],
                        in1=r2_t[:, gs, :],
                    )
                return e_t, r2_t

            def do_pv_blend(c, h, e_t, r2_t):
                qspan = slice(c * CHUNK, (c + 1) * CHUNK)
                # interleave the two PV streams so consecutive matmuls
                # share the same stationary operand (v_aug slice)
                o1 = ps_o1.tile([128, CHUNK], F32, tag="o1", name=f"o1_{c}_{h}")
                o2 = ps_o2.tile([128, CHUNK], F32, tag="o2", name=f"o2_{c}_{h}")
                for kt in range(NKT):
                    nc.tensor.matmul(
                        o1[0:65, :],
                        lhsT=v_s[kt][:, h, :],
                        rhs=e_t[:, kt, :],
                        start=(kt == 0),
                        stop=(kt == NKT - 1),
                    )
                    nc.tensor.matmul(
                        o2[0:65, :],
                        lhsT=v_s[kt][:, h, :],
                        rhs=r2_t[:, kt, :],
                        start=(kt == 0),
                        stop=(kt == NKT - 1),
                    )

                # blend: oT[h] = (wn0/rowsum) * O1 + wn1 * O2
                with nc.allow_low_precision(
                    reason="1/rowsum ~5e-4, fp16 rel eps is plenty"
                ):
                    nc.vector.reciprocal(out=rsinv[64:65, :], in_=o1[64:65, :])
                rb = ps_rb.tile([128, CHUNK], F32, tag="rb", name=f"rb{c}_{h}")
                nc.tensor.matmul(
                    rb[0:65, :],
                    lhsT=wn0_s[:],
                    rhs=rsinv[:],
                    start=True,
                    stop=True,
                )
                rb_sb = blpool.tile(
                    [128, CHUNK], F16, tag="rb_sb", name=f"rbs{c}_{h}"
                )
                nc.scalar.copy(out=rb_sb[0:64, :], in_=rb[0:64, :])
                xb = blpool.tile([128, CHUNK], F16, tag="xb", name=f"xb{c}_{h}")
                nc.vector.tensor_mul(
                    out=xb[0:64, :], in0=o1[0:64, :], in1=rb_sb[0:64, :]
                )
                nc.vector.scalar_tensor_tensor(
                    out=oT[h][0:64, qspan],
                    in0=o2[0:64, :],
                    scalar=float(wn1),
                    in1=xb[0:64, :],
                    op0=mybir.AluOpType.mult,
                    op1=mybir.AluOpType.add,
                )

            def do_proj(c):
                for qt in range(CHUNK // 128):
                    row0 = c * CHUNK + qt * 128
                    part_sb = pspool.tile(
                        [128, C], F16, tag="part", name=f"part{c}_{qt}"
                    )
                    for cc in range(2):
                        csl = slice(cc * 512, (cc + 1) * 512)
                        pp = ps_pp.tile(
                            [128, 512], F32, tag="pp", name=f"pp{c}_{qt}_{cc}"
                        )
                        nc.tensor.matmul(
                            pp[:],
                            lhsT=one_s[:],
                            rhs=bias_s[:, csl],
                            start=True,
                            stop=False,
                        )
                        for h in range(HPC):
                            nc.tensor.matmul(
                                pp[:],
                                lhsT=oT[h][:, row0 : row0 + 128],
                                rhs=wp_s[h][:, csl],
                                start=False,
                                stop=(h == HPC - 1),
                            )
                        nc.scalar.copy(out=part_sb[:, csl], in_=pp[:])
                    nc.sync.dma_start(
                        out=partial_d[row0 : row0 + 128, :], in_=part_sb[:]
                    )

            steps = [(c, h) for c in range(NCH) for h in range(HPC)]
            pending = do_scores(*steps[0])
            for i, (c, h) in enumerate(steps):
                cur = pending
                if i + 1 < len(steps):
                    pending = do_scores(*steps[i + 1])
                do_pv_blend(c, h, *cur)
                if h == HPC - 1:
                    do_proj(c)

            # ---- reduce over the 4 cores of this batch (fp16 partials) ----
            nc.gpsimd.collective_compute(
                "ReduceScatter",
                mybir.AluOpType.add,
                replica_groups=GROUPS,
                ins=[partial_d.opt()],
                outs=[rs_d.opt()],
            )
            nc.sync.dma_start(out=out_ext[:], in_=rs_d[:])

    nc.compile()
    return nc


def _ensure_profile_hook():
    """The container's antenv lacks axon_hooks; recreate it and register
    the ctypes NTFF hook so trace=True yields neuron-profile exec times."""
    import sys
    import types

    try:
        from antenv import axon_hooks  # noqa: F401
    except ImportError:
        import antenv

        mod = types.ModuleType("antenv.axon_hooks")
        _hook = [None]
        mod.set_axon_ntff_profile_hook = lambda h: _hook.__setitem__(0, h)
        mod.get_axon_ntff_profile_hook = lambda: _hook[0]
        sys.modules["antenv.axon_hooks"] = mod
        antenv.axon_hooks = mod
        try:
            from trn_agent_boot.trn_boot import _ntff_profile_via_ctypes

            mod.set_axon_ntff_profile_hook(
                _ntff_profile_via_ctypes("/opt/axon/libaxon_pjrt.so")
            )
        except Exception as e:  # pragma: no cover
            print(f"[kernel] NTFF hook registration failed: {e}")
    # keep profiling artifacts local; the S3 upload has no creds here
    import concourse.bass_utils as bu

    bu.upload_artifacts = lambda tmpdir: tmpdir


def _softmax2(w):
    w = np.asarray(w, np.float64)
    e = np.exp(w - w.max())
    e /= e.sum()
    return float(e[0]), float(e[1])


def kernel(x1, x2, Wq, Wkv, Wproj, bproj, w):
    x1 = np.asarray(x1, np.float32)
    x2 = np.asarray(x2, np.float32)
    Wq = np.asarray(Wq, np.float32)
    Wkv = np.asarray(Wkv, np.float32)
    Wproj = np.asarray(Wproj, np.float32)
    bproj = np.asarray(bproj, np.float32)
    wn0, wn1 = _softmax2(w)

    key = (round(wn0, 9), round(wn1, 9))
    if key not in _CACHE:
        _CACHE[key] = _build(wn0, wn1)
    nc = _CACHE[key]

    scale = Dh ** -0.5
    c_one = np.ones((128, 128), np.float16)
    c_wn0 = np.full((128, 65), wn0, np.float16)

    in_maps = []
    for core in range(NCORES):
        b, g = divmod(core, HPC)
        cols = slice(g * HPC * Dh, (g + 1) * HPC * Dh)
        wp_pad = np.zeros((HPC, 128, C), np.float16)
        for h in range(HPC):
            r0 = g * HPC * Dh + h * Dh
            wp_pad[h, 0:64, :] = Wproj[r0 : r0 + Dh, :].astype(np.float16)
        bias_i = np.zeros((128, C), np.float16)
        if g == 0:
            bias_i[0, :] = bproj.astype(np.float16)
        in_maps.append(
            {
                "x1t": np.ascontiguousarray(x1[b].T).astype(np.float16),
                "x2t": np.ascontiguousarray(x2[b].T).astype(np.float16),
                "wq": (Wq[:, cols] * scale).astype(np.float16),
                "wk": Wkv[:, 0:C][:, cols].astype(np.float16),
                "wv": Wkv[:, C : 2 * C][:, cols].astype(np.float16),
                "wp": wp_pad,
                "biasp": bias_i,
                "c_one": c_one,
                "c_wn0": c_wn0,
            }
        )

    bench = os.environ.get("K_BENCH", "0") == "1"
    if bench:
        _ensure_profile_hook()
    res = run_bass_kernel_spmd(
        nc, in_maps, core_ids=list(range(NCORES)), trace=bench
    )
    if bench:
        kernel.last_exec_ns = res.exec_time_ns
        kernel.last_trace = (
            res.instructions_and_trace[1] if res.instructions_and_trace else None
        )

    full = np.empty((B, N, C), np.float32)
    for b in range(B):
        for r in range(4):
            full[b, r * 512 : (r + 1) * 512, :] = res.results[4 * b + r][
                "out"
            ].astype(np.float32)
    return full


kernel.last_exec_ns = None
kernel.last_trace = None


# revision 18
# speedup vs baseline: 1.2026x; 1.0954x over previous
"""Trainium2 Bass kernel for AdaptiveSparseCrossAttention.

Reference math (B=2, N=2048, C=1024, H=16, Dh=64):
    q  = (x1 @ Wq) [B,H,N,Dh];  k,v = (x2 @ Wkv) [B,H,N,Dh]
    S  = (q * Dh^-0.5) @ k^T                  [B,H,N,N]
    P  = wn0 * softmax(S) + wn1 * relu(S)^2   (wn = softmax(w))
    out = (P @ v).reshape(B,N,C) @ Wproj + bproj

Sharding: 32 (batch, head) pairs over 8 cores -> core i handles batch
b=i//4, heads 4g..4g+3 with g=i%4.  Each core computes a partial
projection [2048,1024]; a ReduceScatter(add) over the 4 cores of each
batch yields 512 distinct output rows per core; the host concatenates.

Device-side layout (per core), all matmuls in (128,128) array mode:
    qT/kT per head: [128, 2048] fp16, Dh values zero-padded to 128 parts
    S^T tile  = kT_slice.T @ qT_chunk   -> PSUM [128 ktoks, 512 q] fp32
    E = exp(S^T) (ScalarE), R2 = (S max 0)*S (VectorE STT) -> fp16 SBUF
    O1 += v_aug.T @ E (v_aug has a ones column -> row 64 = softmax denom)
    O2 += v_aug.T @ R2
    O_hT = (O1 * wn0/rowsum-broadcast) + wn1*O2   (per-head [64, q])
    partial = bias_bcast + sum_h O_hT.T @ Wproj_rows(h)
"""

import os
import numpy as np

import concourse.bass as bass
import concourse.tile as tile
from concourse import bacc, mybir
from concourse.bass_utils import run_bass_kernel_spmd

F16 = mybir.dt.float16
F32 = mybir.dt.float32

B, N, C, H, Dh = 2, 2048, 1024, 16, 64
NCORES = 8
HPC = 4            # heads per core
GROUPS = [[0, 1, 2, 3], [4, 5, 6, 7]]
CHUNK = 512        # q-span processed per (head, chunk) pass
NKT = N // 128     # 16 k-token tiles
NCH = N // CHUNK   # 4 q-chunks
KTG = 2            # k-tiles per S-psum group (exp/relu2 FD = KTG*CHUNK)

RELU2_STT = os.environ.get("K_RELU2", "stt") == "stt"

_CACHE = {}


def _build(wn0: float, wn1: float):
    nc = bacc.Bacc(
        "TRN2", target_bir_lowering=False, debug=False, num_devices=NCORES
    )

    # ---- DRAM parameters (per-core shards fed via in_maps) ----
    x1t = nc.dram_tensor("x1t", [C, N], F16, kind="ExternalInput").ap()
    x2t = nc.dram_tensor("x2t", [C, N], F16, kind="ExternalInput").ap()
    wq = nc.dram_tensor("wq", [C, HPC * Dh], F16, kind="ExternalInput").ap()
    wk = nc.dram_tensor("wk", [C, HPC * Dh], F16, kind="ExternalInput").ap()
    wv = nc.dram_tensor("wv", [C, HPC * Dh], F16, kind="ExternalInput").ap()
    wp = nc.dram_tensor("wp", [HPC, 128, C], F16, kind="ExternalInput").ap()
    biasp = nc.dram_tensor("biasp", [128, C], F16, kind="ExternalInput").ap()
    c_one = nc.dram_tensor("c_one", [128, 128], F16, kind="ExternalInput").ap()
    c_wn0 = nc.dram_tensor("c_wn0", [128, 65], F16, kind="ExternalInput").ap()
    out_ext = nc.dram_tensor(
        "out", [N // 4, C], F16, kind="ExternalOutput"
    ).ap()

    with tile.TileContext(nc) as tc:
        from contextlib import ExitStack

        with ExitStack() as ctx:
            consts = ctx.enter_context(tc.tile_pool(name="consts", bufs=1))
            wpool = ctx.enter_context(tc.tile_pool(name="wpool", bufs=1))
            qkpool = ctx.enter_context(tc.tile_pool(name="qkpool", bufs=1))
            vpool = ctx.enter_context(tc.tile_pool(name="vpool", bufs=1))
            opool = ctx.enter_context(tc.tile_pool(name="opool", bufs=1))
            dram = ctx.enter_context(
                tc.tile_pool(name="dram", bufs=1, space="DRAM")
            )

            ps_s = ctx.enter_context(
                tc.tile_pool(name="ps_s", bufs=2, space="PSUM")
            )
            ps_o1 = ctx.enter_context(
                tc.tile_pool(name="ps_o1", bufs=1, space="PSUM")
            )
            ps_o2 = ctx.enter_context(
                tc.tile_pool(name="ps_o2", bufs=1, space="PSUM")
            )
            ps_rb = ctx.enter_context(
                tc.tile_pool(name="ps_rb", bufs=1, space="PSUM")
            )
            ps_pp = ctx.enter_context(
                tc.tile_pool(name="ps_pp", bufs=1, space="PSUM")
            )

            # ---- persistent SBUF tensors ----
            one_s = consts.tile([128, 128], F16, tag="one")
            wn0_s = consts.tile([128, 65], F16, tag="wn0")
            bias_s = consts.tile([128, C], F16, tag="bias")
            nc.sync.dma_start(out=one_s[:], in_=c_one[:])
            nc.sync.dma_start(out=wn0_s[:], in_=c_wn0[:])
            nc.sync.dma_start(out=bias_s[:], in_=biasp[:])

            wq_s = [wpool.tile([128, HPC * Dh], F16, tag=f"wq{k}", name=f"wq{k}") for k in range(8)]
            wk_s = [wpool.tile([128, HPC * Dh], F16, tag=f"wk{k}", name=f"wk{k}") for k in range(8)]
            wv_s = [wpool.tile([128, HPC * Dh], F16, tag=f"wv{k}", name=f"wv{k}") for k in range(8)]
            wp_s = [wpool.tile([128, C], F16, tag=f"wp{h}", name=f"wp{h}") for h in range(HPC)]
            for k in range(8):
                sl = slice(k * 128, (k + 1) * 128)
                nc.sync.dma_start(out=wq_s[k][:], in_=wq[sl, :])
                nc.sync.dma_start(out=wk_s[k][:], in_=wk[sl, :])
                nc.sync.dma_start(out=wv_s[k][:], in_=wv[sl, :])
            for h in range(HPC):
                nc.sync.dma_start(out=wp_s[h][:], in_=wp[h, :, :])

            # per-head q^T / k^T, zero-padded to K=128
            qT = [qkpool.tile([128, N], F16, tag=f"qT{h}", name=f"qT{h}") for h in range(HPC)]
            kT = [qkpool.tile([128, N], F16, tag=f"kT{h}", name=f"kT{h}") for h in range(HPC)]
            for h in range(HPC):
                nc.vector.memset(qT[h][:], 0.0)
                nc.vector.memset(kT[h][:], 0.0)

            # v with ones column: [128 toks, HPC, 65]
            v_s = [vpool.tile([128, HPC, 65], F16, tag=f"v{t}", name=f"v{t}") for t in range(NKT)]
            for t in range(NKT):
                nc.vector.memset(v_s[t][:, :, 64:65], 1.0)

            # per-head O^T accumulators [128(64 dh + 64 zeros), N] fp16
            oT = [opool.tile([128, N], F16, tag=f"oT{h}", name=f"oT{h}") for h in range(HPC)]
            for h in range(HPC):
                nc.vector.memset(oT[h][:], 0.0)

            partial_d = dram.tile([N, C], F16, name="partial_d")
            rs_d = dram.tile([N // 4, C], F16, name="rs_d")

            # ---- Phase 1: QKV projections ----
            with tc.tile_pool(name="xt", bufs=1) as xpool:
                x1_s = [xpool.tile([128, N], F16, tag=f"x1_{k}", name=f"x1_{k}") for k in range(8)]
                x2_s = [xpool.tile([128, N], F16, tag=f"x2_{k}", name=f"x2_{k}") for k in range(8)]
                for k in range(8):
                    sl = slice(k * 128, (k + 1) * 128)
                    nc.sync.dma_start(out=x1_s[k][:], in_=x1t[sl, :])
                    nc.sync.dma_start(out=x2_s[k][:], in_=x2t[sl, :])

                # qT / kT:  out[h-pair 128, nq 512] = Wq_slice.T @ x1t
                for which, w_s, x_s, dst in (
                    ("q", wq_s, x1_s, qT),
                    ("k", wk_s, x2_s, kT),
                ):
                    for m in range(2):  # head pair (2m, 2m+1)
                        for n in range(4):  # 512-wide q spans
                            pt = ps_s.tile([128, KTG, CHUNK], F32, tag="s")
                            acc = pt[:, 0, :]
                            for k in range(8):
                                nc.tensor.matmul(
                                    acc,
                                    lhsT=w_s[k][:, m * 128 : (m + 1) * 128],
                                    rhs=x_s[k][:, n * 512 : (n + 1) * 512],
                                    start=(k == 0),
                                    stop=(k == 7),
                                )
                            span = slice(n * 512, (n + 1) * 512)
                            nc.scalar.copy(
                                out=dst[2 * m][0:64, span], in_=acc[0:64, :]
                            )
                            nc.scalar.copy(
                                out=dst[2 * m + 1][64:128, span],
                                in_=acc[64:128, :],
                            )

                # v: out[tok 128, HPC*Dh] = x2t_slice.T @ Wv
                for t in range(NKT):
                    pt = ps_s.tile([128, KTG, CHUNK], F32, tag="s")
                    acc = pt[:, 0, 0:256]
                    for k in range(8):
                        nc.tensor.matmul(
                            acc,
                            lhsT=x2_s[k][:, t * 128 : (t + 1) * 128],
                            rhs=wv_s[k][:, 0:256],
                            start=(k == 0),
                            stop=(k == 7),
                        )
                    nc.vector.tensor_copy(
                        out=v_s[t][:, :, 0:64],
                        in_=acc.rearrange("p (h d) -> p h d", h=HPC),
                    )

            # ---- Phase 2: attention + blend, software-pipelined ----
            # S/exp/relu2 of step i+1 are issued before PV/blend of step i,
            # so ScalarE/VectorE chew the next head's scores while the PE
            # runs the current head's PV matmuls.
            epool = ctx.enter_context(tc.tile_pool(name="epool", bufs=2))
            r2pool = ctx.enter_context(tc.tile_pool(name="r2pool", bufs=2))
            blpool = ctx.enter_context(tc.tile_pool(name="blpool", bufs=2))
            rspool = ctx.enter_context(tc.tile_pool(name="rspool", bufs=1))
            pspool = ctx.enter_context(tc.tile_pool(name="pspool", bufs=2))

            rsinv = rspool.tile([128, CHUNK], F16, tag="rsinv")
            nc.vector.memset(rsinv[:], 0.0)

            def do_scores(c, h):
                """S^T matmuls + exp + relu^2 for (chunk c, head h)."""
                qspan = slice(c * CHUNK, (c + 1) * CHUNK)
                e_t = epool.tile([128, NKT, CHUNK], F16, tag="e", name=f"e{c}_{h}")
                r2_t = r2pool.tile(
                    [128, NKT, CHUNK], F16, tag="r2", name=f"r2{c}_{h}"
                )
                for g in range(NKT // KTG):
                    s_ps = ps_s.tile(
                        [128, KTG, CHUNK], F32, tag="s", name=f"s{c}_{h}_{g}"
                    )
                    for j in range(KTG):
                        kt = g * KTG + j
                        nc.tensor.matmul(
                            s_ps[:, j, :],
                            lhsT=kT[h][:, kt * 128 : (kt + 1) * 128],
                            rhs=qT[h][:, qspan],
                            start=True,
                            stop=True,
                        )
                    gs = slice(g * KTG, (g + 1) * KTG)
                    nc.scalar.activation(
                        out=e_t[:, gs, :],
                        in_=s_ps[:],
                        func=mybir.ActivationFunctionType.Exp,
                    )
                    # relu(S)^2: max into SBUF fp16, then square in place
                    # (walrus forbids two PSUM operands on one DVE op).
                    # Work is spread across ScalarE/VectorE/GpSimd to
                    # balance engines (DVE reads fp32 PSUM at 1x only).
                    if g % 8 in (2, 5, 7):
                        nc.scalar.activation(
                            out=r2_t[:, gs, :],
                            in_=s_ps[:],
                            func=mybir.ActivationFunctionType.Relu,
                        )
                    else:
                        nc.vector.tensor_scalar_max(
                            out=r2_t[:, gs, :], in0=s_ps[:], scalar1=0.0
                        )
                    sq_eng = nc.vector  # gpsimd TT crashed NRT; keep on DVE
                    sq_eng.tensor_mul(
                        out=r2_t[:, gs, :],
                        in0=r2_t[:, gs, :],
                        in1=r2_t[:, gs, :],
                    )
                return e_t, r2_t

            def do_pv_blend(c, h, e_t, r2_t):
                qspan = slice(c * CHUNK, (c + 1) * CHUNK)
                # interleave the two PV streams so consecutive matmuls
                # share the same stationary operand (v_aug slice)
                o1 = ps_o1.tile([128, CHUNK], F32, tag="o1", name=f"o1_{c}_{h}")
                o2 = ps_o2.tile([128, CHUNK], F32, tag="o2", name=f"o2_{c}_{h}")
                for kt in range(NKT):
                    nc.tensor.matmul(
                        o1[0:65, :],
                        lhsT=v_s[kt][:, h, :],
                        rhs=e_t[:, kt, :],
                        start=(kt == 0),
                        stop=(kt == NKT - 1),
                    )
                    nc.tensor.matmul(
                        o2[0:65, :],
                        lhsT=v_s[kt][:, h, :],
                        rhs=r2_t[:, kt, :],
                        start=(kt == 0),
                        stop=(kt == NKT - 1),
                    )

                # blend: oT[h] = (wn0/rowsum) * O1 + wn1 * O2
                with nc.allow_low_precision(
                    reason="1/rowsum ~5e-4, fp16 rel eps is plenty"
                ):
                    nc.vector.reciprocal(out=rsinv[64:65, :], in_=o1[64:65, :])
                rb = ps_rb.tile([128, CHUNK], F32, tag="rb", name=f"rb{c}_{h}")
                nc.tensor.matmul(
                    rb[0:65, :],
                    lhsT=wn0_s[:],
                    rhs=rsinv[:],
                    start=True,
                    stop=True,
                )
                rb_sb = blpool.tile(
                    [128, CHUNK], F16, tag="rb_sb", name=f"rbs{c}_{h}"
                )
                nc.scalar.copy(out=rb_sb[0:64, :], in_=rb[0:64, :])
                xb = blpool.tile([128, CHUNK], F16, tag="xb", name=f"xb{c}_{h}")
                nc.vector.tensor_mul(
                    out=xb[0:64, :], in0=o1[0:64, :], in1=rb_sb[0:64, :]
                )
                nc.vector.scalar_tensor_tensor(
                    out=oT[h][0:64, qspan],
                    in0=o2[0:64, :],
                    scalar=float(wn1),
                    in1=xb[0:64, :],
                    op0=mybir.AluOpType.mult,
                    op1=mybir.AluOpType.add,
                )

            def do_proj(c):
                for qt in range(CHUNK // 128):
                    row0 = c * CHUNK + qt * 128
                    part_sb = pspool.tile(
                        [128, C], F16, tag="part", name=f"part{c}_{qt}"
                    )
                    for cc in range(2):
                        csl = slice(cc * 512, (cc + 1) * 512)
                        pp = ps_pp.tile(
                            [128, 512], F32, tag="pp", name=f"pp{c}_{qt}_{cc}"
                        )
                        nc.tensor.matmul(
                            pp[:],
                            lhsT=one_s[:],
                            rhs=bias_s[:, csl],
                            start=True,
                            stop=False,
                        )
                        for h in range(HPC):
                            nc.tensor.matmul(
                                pp[:],
                                lhsT=oT[h][:, row0 : row0 + 128],
                                rhs=wp_s[h][:, csl],
                                start=False,
                                stop=(h == HPC - 1),
                            )
                        nc.scalar.copy(out=part_sb[:, csl], in_=pp[:])
                    nc.sync.dma_start(
                        out=partial_d[row0 : row0 + 128, :], in_=part_sb[:]
                    )

            def do_rs(half):
                # reduce rows [half*1024, half*1024+1024) over the 4 cores
                # of this batch group (fp16).  Rank r receives global rows
                # half*1024 + r*256 + [0, 256).
                r0, r1 = half * (N // 2), (half + 1) * (N // 2)
                o0, o1_ = half * (N // 8), (half + 1) * (N // 8)
                nc.gpsimd.collective_compute(
                    "ReduceScatter",
                    mybir.AluOpType.add,
                    replica_groups=GROUPS,
                    ins=[partial_d[r0:r1, :]],
                    outs=[rs_d[o0:o1_, :]],
                )
                nc.sync.dma_start(
                    out=out_ext[o0:o1_, :], in_=rs_d[o0:o1_, :]
                )

            steps = [(c, h) for c in range(NCH) for h in range(HPC)]
            pending = do_scores(*steps[0])
            for i, (c, h) in enumerate(steps):
                cur = pending
                if i + 1 < len(steps):
                    pending = do_scores(*steps[i + 1])
                do_pv_blend(c, h, *cur)
                if h == HPC - 1:
                    do_proj(c)
                    if c == NCH // 2 - 1:
                        do_rs(0)  # first half overlaps second-half compute
                    elif c == NCH - 1:
                        do_rs(1)

    nc.compile()
    return nc


def _ensure_profile_hook():
    """The container's antenv lacks axon_hooks; recreate it and register
    the ctypes NTFF hook so trace=True yields neuron-profile exec times."""
    import sys
    import types

    try:
        from antenv import axon_hooks  # noqa: F401
    except ImportError:
        import antenv

        mod = types.ModuleType("antenv.axon_hooks")
        _hook = [None]
        mod.set_axon_ntff_profile_hook = lambda h: _hook.__setitem__(0, h)
        mod.get_axon_ntff_profile_hook = lambda: _hook[0]
        sys.modules["antenv.axon_hooks"] = mod
        antenv.axon_hooks = mod
        try:
            from trn_agent_boot.trn_boot import _ntff_profile_via_ctypes

            mod.set_axon_ntff_profile_hook(
                _ntff_profile_via_ctypes("/opt/axon/libaxon_pjrt.so")
            )
        except Exception as e:  # pragma: no cover
            print(f"[kernel] NTFF hook registration failed: {e}")
    # keep profiling artifacts local; the S3 upload has no creds here
    import concourse.bass_utils as bu

    bu.upload_artifacts = lambda tmpdir: tmpdir


def _softmax2(w):
    w = np.asarray(w, np.float64)
    e = np.exp(w - w.max())
    e /= e.sum()
    return float(e[0]), float(e[1])


def kernel(x1, x2, Wq, Wkv, Wproj, bproj, w):
    x1 = np.asarray(x1, np.float32)
    x2 = np.asarray(x2, np.float32)
    Wq = np.asarray(Wq, np.float32)
    Wkv = np.asarray(Wkv, np.float32)
    Wproj = np.asarray(Wproj, np.float32)
    bproj = np.asarray(bproj, np.float32)
    wn0, wn1 = _softmax2(w)

    key = (round(wn0, 9), round(wn1, 9))
    if key not in _CACHE:
        _CACHE[key] = _build(wn0, wn1)
    nc = _CACHE[key]

    scale = Dh ** -0.5
    c_one = np.ones((128, 128), np.float16)
    c_wn0 = np.full((128, 65), wn0, np.float16)

    in_maps = []
    for core in range(NCORES):
        b, g = divmod(core, HPC)
        cols = slice(g * HPC * Dh, (g + 1) * HPC * Dh)
        wp_pad = np.zeros((HPC, 128, C), np.float16)
        for h in range(HPC):
            r0 = g * HPC * Dh + h * Dh
            wp_pad[h, 0:64, :] = Wproj[r0 : r0 + Dh, :].astype(np.float16)
        bias_i = np.zeros((128, C), np.float16)
        if g == 0:
            bias_i[0, :] = bproj.astype(np.float16)
        in_maps.append(
            {
                "x1t": np.ascontiguousarray(x1[b].T).astype(np.float16),
                "x2t": np.ascontiguousarray(x2[b].T).astype(np.float16),
                "wq": (Wq[:, cols] * scale).astype(np.float16),
                "wk": Wkv[:, 0:C][:, cols].astype(np.float16),
                "wv": Wkv[:, C : 2 * C][:, cols].astype(np.float16),
                "wp": wp_pad,
                "biasp": bias_i,
                "c_one": c_one,
                "c_wn0": c_wn0,
            }
        )

    bench = os.environ.get("K_BENCH", "0") == "1"
    if bench:
        _ensure_profile_hook()
    res = run_bass_kernel_spmd(
        nc, in_maps, core_ids=list(range(NCORES)), trace=bench
    )
    if bench:
        kernel.last_exec_ns = res.exec_time_ns
        kernel.last_trace = (
            res.instructions_and_trace[1] if res.instructions_and_trace else None
        )

    full = np.empty((B, N, C), np.float32)
    for b in range(B):
        for r in range(4):
            o = res.results[4 * b + r]["out"].astype(np.float32)
            for half in range(2):
                dst0 = half * (N // 2) + r * (N // 8)
                full[b, dst0 : dst0 + N // 8, :] = o[
                    half * (N // 8) : (half + 1) * (N // 8), :
                ]
    return full


kernel.last_exec_ns = None
kernel.last_trace = None
